# revision 40
# baseline (speedup 1.0000x reference)
"""Trainium2 Bass kernel for nn_LlamaAttention_48816598286577.

Llama attention with block-streaming sparse mask (sink=1 block, local
window=8 blocks, BLOCK=128), B=1 S=2048 H=4096, 32 q heads / 8 kv heads,
head_dim 128, non-interleaved RoPE.

Sharding: tensor-parallel over heads across 8 cores (4 q heads + 1 kv
head per core). All compute in bf16 (PSUM accumulates f32): full PE rate
at any matmul width, half the DMA/collective traffic of f32.

Structure:
- Host pre-swizzles hid/weights chunk-major so every DMA has >=1KB
  contiguous lines per partition; weight and quarter-0 hid DMAs are
  interleaved so the first projection matmul issues within ~10us.
- Two tiny warm-up AllGathers absorb the ~100us CC-stream barrier under
  phase 1.
- V natural blocks carry an appended ones-column and the PV matmul uses
  e_t (exp scores) as stationary -> PSUM holds natural-layout
  [q, d | rowsum]; softmax denominator falls out of the matmul chain and
  normalization is a per-partition tensor_scalar_mul.
- o_proj for chunk i-3 is interleaved between the attention heads of
  chunk i (ag_sb DMAs prefetched one head earlier) so the PE never
  stalls on the AllGather or on ACT exp lag.
- Phase-2 SBUF pools are allocated below phase-1 pools (not reused) so
  the phase boundary carries no write-after-read waits.
"""

import functools
import numpy as np

import concourse.bass as bass
import concourse.mybir as mybir
import concourse.tile as tile
from concourse import bacc
from concourse.bass_utils import run_bass_kernel_spmd

# problem constants (hardcoded per contract)
B, S, H = 1, 2048, 4096
NQ, NKV, HD = 32, 8, 128
BLOCK = 128
NBLK = S // BLOCK          # 16
SINK_BLOCKS = 1
LOCAL_BLOCKS = 8
ROPE_BASE = 10000.0
N_CORES = 8
HQ = NQ // N_CORES         # 4 q heads per core
DQ = HQ * HD               # 512 q columns per core
SCALE = 1.0 / float(np.sqrt(HD))

KC = H // 128              # 32 contraction chunks for projections
NQUART = 4                 # S split into 4 quarters of 512 for projections
QW = S // NQUART           # 512
NPAIR = NBLK // 2          # 8 query pairs of 256
NPRE = 24                  # quarter-0 hid chunks preloaded with the weights

F32 = mybir.dt.float32
BF16 = mybir.dt.bfloat16

VB = 129                   # v-block stride in vNat (128 v cols + ones col)
DEPTH = 3                  # o_proj pipeline depth in chunks


def _pair_blocks(i: int):
    """Key blocks for query pair i with per-block subblock coverage.

    Returns list of (j, left, right): left/right = whether q-block 2i /
    2i+1 attends to key block j (causal + sink-or-local, block level).
    """
    out = []
    for j in range(2 * i + 2):
        left = j <= 2 * i and (2 * i - j < LOCAL_BLOCKS or j < SINK_BLOCKS)
        right = j <= 2 * i + 1 and (2 * i + 1 - j < LOCAL_BLOCKS or j < SINK_BLOCKS)
        if left or right:
            out.append((j, left, right))
    return out


def build_nc():
    nc = bacc.Bacc(
        "TRN2", target_bir_lowering=False, debug=False, num_devices=N_CORES
    )
    # chunk-major swizzled inputs (see _run): index [p, c*W + x] holds
    # original [c*128 + p, x].
    hid_sw = nc.dram_tensor("hid_sw", [128, KC * S], BF16, kind="ExternalInput").ap()
    wq_sw = nc.dram_tensor("wq_sw", [128, KC * DQ], BF16, kind="ExternalInput").ap()
    wk_sw = nc.dram_tensor("wk_sw", [128, KC * HD], BF16, kind="ExternalInput").ap()
    wv_sw = nc.dram_tensor("wv_sw", [128, KC * HD], BF16, kind="ExternalInput").ap()
    wo_sw = nc.dram_tensor("wo_sw", [128, KC * DQ], BF16, kind="ExternalInput").ap()
    cosF = nc.dram_tensor("cosF", [128, S], F32, kind="ExternalInput").ap()
    sinS = nc.dram_tensor("sinS", [128, S], F32, kind="ExternalInput").ap()
    tri = nc.dram_tensor("tri", [128, 128], BF16, kind="ExternalInput").ap()
    eye = nc.dram_tensor("eye", [128, 128], BF16, kind="ExternalInput").ap()
    out = nc.dram_tensor("out", [S, DQ], F32, kind="ExternalOutput").ap()

    with tile.TileContext(nc) as tc:
        with (
            tc.tile_pool(name="persist", bufs=1) as pp,
            tc.tile_pool(name="dram", bufs=1, space="DRAM") as dramp,
        ):
            # ---- persistent SBUF state
            qTr = [
                [
                    pp.tile([128, QW], BF16, tag=f"qTr{h}_{nq}", name=f"qTr{h}_{nq}")
                    for nq in range(NQUART)
                ]
                for h in range(HQ)
            ]
            kTr = [
                pp.tile([128, QW], BF16, tag=f"kTr{nq}", name=f"kTr{nq}")
                for nq in range(NQUART)
            ]
            # natural-layout V, 4 blocks per quarter, each [128, 129]
            # (last col = ones -> PV matmul also emits the softmax rowsum)
            vNat = [
                pp.tile([128, 4 * VB], BF16, tag=f"vNat{nq}", name=f"vNat{nq}")
                for nq in range(NQUART)
            ]
            tri_sb = pp.tile([128, 128], BF16, tag="tri", name="tri_sb")
            eye_sb = pp.tile([128, 128], BF16, tag="eye", name="eye_sb")
            wq_sb = pp.tile([128, KC * DQ], BF16, tag="wq", name="wq_sb")
            wk_sb = pp.tile([128, KC * HD], BF16, tag="wk", name="wk_sb")
            wv_sb = pp.tile([128, KC * HD], BF16, tag="wv", name="wv_sb")
            wo_sb = pp.tile([128, KC * DQ], BF16, tag="wo", name="wo_sb")

            # ---- DRAM collective buffers
            ag_ins = [
                dramp.tile([DQ, 256], BF16, tag=f"agin{c}", name=f"agin{c}")
                for c in range(NPAIR)
            ]
            ag_outs = [
                dramp.tile(
                    [H, 256], BF16, tag=f"agout{c}", name=f"agout{c}",
                    addr_space="Shared",
                )
                for c in range(NPAIR)
            ]

            # Warm up the CC stream immediately: the first collective pays a
            # ~100us all-core barrier + stream setup; two tiny AllGathers up
            # front let that overlap phase 1 instead of stalling o_proj.
            warm_in = dramp.tile([128, 8], BF16, tag="win", name="warm_in")
            warm_sb = pp.tile([128, 8], BF16, tag="wsb", name="warm_sb")
            nc.vector.memset(warm_sb[:], 0.0)
            nc.sync.dma_start(warm_in[:], warm_sb[:])
            warm_outs = [
                dramp.tile(
                    [N_CORES * 128, 8], BF16, tag=f"wout{w}", name=f"warm_out{w}",
                    addr_space="Shared",
                )
                for w in range(2)
            ]
            for w in range(2):
                nc.gpsimd.collective_compute(
                    "AllGather",
                    mybir.AluOpType.bypass,
                    replica_groups=[list(range(N_CORES))],
                    ins=[warm_in.opt()],
                    outs=[warm_outs[w].opt()],
                )

            nc.sync.dma_start(eye_sb[:], eye[:])
            nc.sync.dma_start(tri_sb[:], tri[:])
            # ones columns of vNat (written once, before any transposes land)
            for nq in range(NQUART):
                for b in range(4):
                    nc.vector.memset(vNat[nq][:, b * VB + 128 : b * VB + 129], 1.0)

            # Phase-2 SBUF pools first: they live below the phase-1 pools so
            # the phase boundary has no SBUF reuse hazards.
            with (
                tc.tile_pool(name="p2_e", bufs=4) as ep,
                tc.tile_pool(name="p2_sb", bufs=2) as asb,
                tc.tile_pool(name="p2_ag", bufs=16) as agp,
                tc.tile_pool(name="p2_ev", bufs=2) as evp,
            ):
                # ============= Phase 1: QKV projections + RoPE + V layout
                with (
                    tc.tile_pool(name="p1_stream", bufs=8) as stp,
                    tc.tile_pool(name="p1_small", bufs=2) as sp,
                    tc.tile_pool(name="p1_ps", bufs=1, space="PSUM") as pspp,
                    tc.tile_pool(name="tr_ps", bufs=2, space="PSUM") as trpp,
                ):
                    # qkv weights in c-chunk order interleaved with the
                    # quarter-0 hid stream; the first pieces are single
                    # chunks so the first matmul issues within ~10us
                    bounds = [0, 1, 2, 4, 6, 8] + list(range(12, KC + 1, 4))
                    pieces = list(zip(bounds[:-1], bounds[1:]))
                    hid_q0 = {}
                    for (a, b) in pieces:
                        if a == 0:
                            for st in range(4):
                                nc.sync.dma_start(
                                    wq_sb[:, st * 128 : (st + 1) * 128],
                                    wq_sw[:, st * 128 : (st + 1) * 128],
                                )
                        else:
                            nc.sync.dma_start(
                                wq_sb[:, a * DQ : b * DQ], wq_sw[:, a * DQ : b * DQ]
                            )
                        nc.sync.dma_start(
                            wk_sb[:, a * HD : b * HD], wk_sw[:, a * HD : b * HD]
                        )
                        nc.sync.dma_start(
                            wv_sb[:, a * HD : b * HD], wv_sw[:, a * HD : b * HD]
                        )
                        for c in range(a, min(b, NPRE)):
                            hc = stp.tile(
                                [128, QW], BF16, tag=f"hid0_{c}",
                                name=f"hid0_{c}", bufs=1,
                            )
                            nc.sync.dma_start(hc[:], hid_sw[:, c * S : c * S + QW])
                            hid_q0[c] = hc

                    for nq in range(NQUART):
                        ncols = slice(nq * QW, (nq + 1) * QW)
                        cos_sb = sp.tile([128, QW], F32, tag="cos", name="cos_sb")
                        sin_sb = sp.tile([128, QW], F32, tag="sin", name="sin_sb")
                        nc.sync.dma_start(cos_sb[:], cosF[:, ncols])
                        nc.sync.dma_start(sin_sb[:], sinS[:, ncols])

                        ps_q = [
                            pspp.tile([128, QW], F32, tag=f"psq{h}", name=f"psq{h}")
                            for h in range(HQ)
                        ]
                        ps_k = pspp.tile([128, QW], F32, tag="psk", name="ps_k")
                        ps_v = pspp.tile([128, QW], F32, tag="psv", name="ps_v")
                        for c in range(KC):
                            if nq == 0 and c < NPRE:
                                hid_c = hid_q0.pop(c)
                            else:
                                hid_c = stp.tile(
                                    [128, QW], BF16, tag="hid", name="hid_c"
                                )
                                nc.sync.dma_start(
                                    hid_c[:],
                                    hid_sw[:, c * S + nq * QW : c * S + (nq + 1) * QW],
                                )
                            st, sp_ = (c == 0), (c == KC - 1)
                            for h in range(HQ):
                                nc.tensor.matmul(
                                    ps_q[h][:],
                                    wq_sb[:, c * DQ + h * HD : c * DQ + (h + 1) * HD],
                                    hid_c[:],
                                    start=st,
                                    stop=sp_,
                                )
                            nc.tensor.matmul(
                                ps_k[:], wk_sb[:, c * HD : (c + 1) * HD], hid_c[:],
                                start=st, stop=sp_,
                            )
                            nc.tensor.matmul(
                                ps_v[:], wv_sb[:, c * HD : (c + 1) * HD], hid_c[:],
                                start=st, stop=sp_,
                            )

                        # V: evacuate to bf16, then 4 PE transposes into vNat
                        # (XBAR dma transpose corrupts unaligned dsts)
                        vT_q = sp.tile([128, QW], BF16, tag="vTq", name="vT_q")
                        nc.vector.tensor_copy(vT_q[:], ps_v[:])
                        for b in range(4):
                            tr = trpp.tile([128, 128], BF16, tag="tr", name="tr")
                            nc.tensor.transpose(
                                tr[:], vT_q[:, b * 128 : (b + 1) * 128], eye_sb[:]
                            )
                            nc.vector.tensor_copy(
                                vNat[nq][:, b * VB : b * VB + 128], tr[:]
                            )

                        # RoPE: dst = ps*cos + swap(ps)*sin. PSUM-reading ops
                        # (ACT copy + cos-mul) hoisted for all heads first so
                        # the PSUM banks free quickly. Temps in bf16.
                        srcs = [(ps_k, kTr[nq])] + [
                            (ps_q[h], qTr[h][nq]) for h in range(HQ)
                        ]
                        raws, t1s = [], []
                        for hi, (ps_x, _) in enumerate(srcs):
                            raw = sp.tile(
                                [128, QW], BF16, tag=f"raw{hi}", name=f"raw{hi}",
                                bufs=1,
                            )
                            t1 = sp.tile(
                                [128, QW], BF16, tag=f"t1_{hi}", name=f"t1_{hi}",
                                bufs=1,
                            )
                            nc.scalar.activation(
                                raw[:], ps_x[:], mybir.ActivationFunctionType.Copy
                            )
                            nc.vector.tensor_mul(t1[:], ps_x[:], cos_sb[:])
                            raws.append(raw)
                            t1s.append(t1)
                        for hi, (ps_x, dstT) in enumerate(srcs):
                            raw, t1 = raws[hi], t1s[hi]
                            swp = sp.tile(
                                [128, QW], BF16, tag=f"swp{hi}", name=f"swp{hi}",
                                bufs=1,
                            )
                            nc.sync.dma_start(swp[0:64, :], raw[64:128, :])
                            nc.sync.dma_start(swp[64:128, :], raw[0:64, :])
                            t2 = sp.tile([128, QW], BF16, tag="t2", name="t2")
                            nc.vector.tensor_mul(t2[:], swp[:], sin_sb[:])
                            nc.vector.tensor_add(dstT[:], t1[:], t2[:])

                        # trickle in wo while phase-1 compute runs
                        w = KC * DQ // NQUART
                        nc.sync.dma_start(
                            wo_sb[:, nq * w : (nq + 1) * w],
                            wo_sw[:, nq * w : (nq + 1) * w],
                        )

                # ============= Phase 2: attention + AllGather + o_proj,
                # software-pipelined: oproj(i-DEPTH) inside chunk i.
                with (
                    tc.tile_pool(name="s_ps", bufs=2, space="PSUM") as spsp,
                    tc.tile_pool(name="o_ps", bufs=2, space="PSUM") as opsp,
                    tc.tile_pool(name="op_ps", bufs=1, space="PSUM") as oppp,
                    tc.tile_pool(name="t_ps", bufs=2, space="PSUM") as trp2,
                ):
                    oproj_ps = {}
                    oproj_dmas = {}

                    def oproj_dma(i, sl):
                        """Prefetch the ag_sb tiles for o_proj slice sl."""
                        tiles = []
                        for c in range(8 * sl, 8 * sl + 8):
                            ag_sb = agp.tile(
                                [128, 256], BF16, tag="ag_sb", name="ag_sb"
                            )
                            nc.sync.dma_start(
                                ag_sb[:], ag_outs[i][c * 128 : (c + 1) * 128, :]
                            )
                            tiles.append(ag_sb)
                        oproj_dmas[(i, sl)] = tiles

                    def oproj_mm(i, sl):
                        """o_proj contraction chunks [8*sl, 8*sl+8) of chunk i."""
                        if sl == 0:
                            oproj_ps[i] = [
                                oppp.tile(
                                    [128, DQ], F32, tag=f"op{sb}", name=f"op{sb}"
                                )
                                for sb in range(2)
                            ]
                        ps01 = oproj_ps[i]
                        tiles = oproj_dmas.pop((i, sl))
                        for ci, c in enumerate(range(8 * sl, 8 * sl + 8)):
                            ag_sb = tiles[ci]
                            for sb in range(2):
                                nc.tensor.matmul(
                                    ps01[sb][:],
                                    ag_sb[:, sb * 128 : (sb + 1) * 128],
                                    wo_sb[:, c * DQ : (c + 1) * DQ],
                                    start=(c == 0),
                                    stop=(c == KC - 1),
                                )

                    def oproj_finish(i):
                        ps01 = oproj_ps.pop(i)
                        q0 = i * 256
                        for sb in range(2):
                            ev = evp.tile([128, DQ], F32, tag="ev", name="ev")
                            nc.vector.tensor_copy(ev[:], ps01[sb][:])
                            nc.sync.dma_start(
                                out[q0 + sb * 128 : q0 + (sb + 1) * 128, :], ev[:]
                            )

                    def attn_scores(i, h):
                        q0 = i * 256
                        qq = q0 // QW          # quarter holding this pair
                        qbase = q0 - qq * QW
                        blocks = _pair_blocks(i)
                        widths = [
                            (128 if not (l and r) else 256) for (_, l, r) in blocks
                        ]
                        offs = list(np.cumsum([0] + widths))
                        e_t = ep.tile([128, 2304], BF16, tag="e", name="e_t")

                        # scores in 512-col PSUM groups -> exp -> e_t
                        g = 0
                        while g < len(blocks):
                            g_end = g
                            gw = 0
                            while g_end < len(blocks) and gw + widths[g_end] <= 512:
                                gw += widths[g_end]
                                g_end += 1
                            s_grp = spsp.tile([128, 512], F32, tag="sg", name="s_grp")
                            for bi in range(g, g_end):
                                j, l, r = blocks[bi]
                                qs = qbase if l else qbase + 128
                                w = widths[bi]
                                o = offs[bi] - offs[g]
                                nc.tensor.matmul(
                                    s_grp[:, o : o + w],
                                    kTr[j // 4][:, (j % 4) * 128 : (j % 4 + 1) * 128],
                                    qTr[h][qq][:, qs : qs + w],
                                    start=True,
                                    stop=True,
                                )
                            nc.scalar.activation(
                                e_t[:, offs[g] : offs[g] + gw],
                                s_grp[:, 0:gw],
                                mybir.ActivationFunctionType.Exp,
                                scale=SCALE,
                            )
                            g = g_end

                        # in-block causal masks on the two diagonal blocks
                        for bi, (j, l, r) in enumerate(blocks):
                            if j == 2 * i:
                                nc.vector.tensor_mul(
                                    e_t[:, offs[bi] : offs[bi] + 128],
                                    e_t[:, offs[bi] : offs[bi] + 128],
                                    tri_sb[:],
                                )
                            elif j == 2 * i + 1:
                                o = offs[bi] + (widths[bi] - 128)
                                nc.vector.tensor_mul(
                                    e_t[:, o : o + 128],
                                    e_t[:, o : o + 128],
                                    tri_sb[:],
                                )
                        return (i, h, blocks, widths, offs, e_t)

                    def attn_pv(ctx):
                        i, h, blocks, widths, offs, e_t = ctx
                        # fused PV + rowsum: out_nat [q, 129] per subblock.
                        # One PSUM accumulation group may be pending per zero
                        # region: left group runs to completion, then right.
                        o_nat = opsp.tile([128, 2 * VB], F32, tag="on", name="o_nat")
                        nL = sum(1 for (_, l, _) in blocks if l)
                        nR = sum(1 for (_, _, r) in blocks if r)
                        cL = cR = 0
                        for bi, (j, l, r) in enumerate(blocks):
                            if not l:
                                continue
                            mv = vNat[j // 4][:, (j % 4) * VB : (j % 4) * VB + VB]
                            nc.tensor.matmul(
                                o_nat[:, 0:VB],
                                e_t[:, offs[bi] : offs[bi] + 128],
                                mv,
                                start=(cL == 0),
                                stop=(cL == nL - 1),
                            )
                            cL += 1
                        for bi, (j, l, r) in enumerate(blocks):
                            if not r:
                                continue
                            mv = vNat[j // 4][:, (j % 4) * VB : (j % 4) * VB + VB]
                            o = offs[bi] + (widths[bi] - 128)
                            nc.tensor.matmul(
                                o_nat[:, VB : 2 * VB],
                                e_t[:, o : o + 128],
                                mv,
                                start=(cR == 0),
                                stop=(cR == nR - 1),
                            )
                            cR += 1

                        # normalize per q row, transpose to [d, q] on the PE,
                        # ship to the AllGather input buffer
                        r_sb = asb.tile([128, 2], F32, tag="r", name="r_sb", bufs=4)
                        nc.vector.reciprocal(r_sb[:, 0:1], o_nat[:, 128:129])
                        nc.vector.reciprocal(
                            r_sb[:, 1:2], o_nat[:, 2 * VB - 1 : 2 * VB]
                        )
                        at_nat = asb.tile(
                            [128, 256], BF16, tag="an", name="at_nat", bufs=8
                        )
                        nc.vector.tensor_scalar_mul(
                            at_nat[:, 0:128], o_nat[:, 0:128], r_sb[:, 0:1]
                        )
                        nc.vector.tensor_scalar_mul(
                            at_nat[:, 128:256], o_nat[:, VB : VB + 128], r_sb[:, 1:2]
                        )
                        trT = trp2.tile([128, 256], BF16, tag="trT", name="trT")
                        nc.tensor.transpose(
                            trT[:, 0:128], at_nat[:, 0:128], eye_sb[:]
                        )
                        nc.tensor.transpose(
                            trT[:, 128:256], at_nat[:, 128:256], eye_sb[:]
                        )
                        at_cT = asb.tile(
                            [128, 256], BF16, tag="at", name="at_cT", bufs=8
                        )
                        nc.vector.tensor_copy(at_cT[:], trT[:])
                        nc.sync.dma_start(
                            ag_ins[i][h * 128 : (h + 1) * 128, :], at_cT[:]
                        )

                    for i in range(NPAIR):
                        if i < DEPTH:
                            # no o_proj filler yet: batch all heads' scores
                            # first so ACT exp pipelines ahead of the PVs
                            ctxs = [attn_scores(i, h) for h in range(HQ)]
                            for ctx in ctxs:
                                attn_pv(ctx)
                        else:
                            for h in range(HQ):
                                oproj_dma(i - DEPTH, h)
                                ctx = attn_scores(i, h)
                                attn_pv(ctx)
                                oproj_mm(i - DEPTH, h)
                        nc.gpsimd.collective_compute(
                            "AllGather",
                            mybir.AluOpType.bypass,
                            replica_groups=[list(range(N_CORES))],
                            ins=[ag_ins[i].opt()],
                            outs=[ag_outs[i].opt()],
                        )
                        if i >= DEPTH:
                            oproj_finish(i - DEPTH)
                    for i in range(NPAIR - DEPTH, NPAIR):
                        oproj_dma(i, 0)
                        for sl in range(4):
                            if sl < 3:
                                oproj_dma(i, sl + 1)
                            oproj_mm(i, sl)
                        oproj_finish(i)

    nc.compile()
    return nc


@functools.lru_cache(maxsize=1)
def _cached_nc():
    return build_nc()


def _tables():
    pos = np.arange(S, dtype=np.float64)
    inv = 1.0 / (ROPE_BASE ** (np.arange(0, HD, 2, dtype=np.float64) / HD))  # [64]
    f = inv[:, None] * pos[None, :]                   # [64, S]
    cos = np.cos(f).astype(np.float32)
    sin = np.sin(f).astype(np.float32)
    cosF = np.concatenate([cos, cos], axis=0)         # [128, S]
    sinS = np.concatenate([-sin, sin], axis=0)        # [128, S]
    k_idx = np.arange(128)[:, None]
    q_idx = np.arange(128)[None, :]
    tri = (k_idx <= q_idx).astype(np.float32)         # [k, q] causal in-block
    return cosF, sinS, tri


def _swz(w: np.ndarray, bf16) -> np.ndarray:
    """[KC*128, W] -> chunk-major [128, KC*W] bf16."""
    kc, w_ = w.shape[0] // 128, w.shape[1]
    return np.ascontiguousarray(
        w.reshape(kc, 128, w_).transpose(1, 0, 2).reshape(128, kc * w_)
    ).astype(bf16)


def _run(hidden_states, wq, wk, wv, wo, **run_kwargs):
    nc = _cached_nc()
    bf16 = mybir.dt.np(BF16)
    # hid_sw[p, c*S + s] = hidden[s, c*128 + p]
    hid2 = np.asarray(hidden_states, dtype=np.float32).reshape(S, H)
    hid_sw = np.ascontiguousarray(
        hid2.reshape(S, KC, 128).transpose(2, 1, 0).reshape(128, KC * S)
    ).astype(bf16)
    cosF, sinS, tri = _tables()
    in_maps = []
    for c in range(N_CORES):
        in_maps.append(
            {
                "hid_sw": hid_sw,
                "wq_sw": _swz(wq[:, c * DQ : (c + 1) * DQ], bf16),
                "wk_sw": _swz(wk[:, c * HD : (c + 1) * HD], bf16),
                "wv_sw": _swz(wv[:, c * HD : (c + 1) * HD], bf16),
                "wo_sw": _swz(wo[:, c * DQ : (c + 1) * DQ], bf16),
                "cosF": cosF,
                "sinS": sinS,
                "tri": tri.astype(bf16),
                "eye": np.eye(128, dtype=np.float32).astype(bf16),
            }
        )
    res = run_bass_kernel_spmd(
        nc, in_maps, core_ids=list(range(N_CORES)), **run_kwargs
    )
    full = np.concatenate(
        [res.results[r]["out"] for r in range(N_CORES)], axis=1
    )
    return full.reshape(B, S, H).astype(np.float32), res


def kernel(hidden_states, wq, wk, wv, wo):
    out, _ = _run(hidden_states, wq, wk, wv, wo)
    return out
